# revision 2
# baseline (speedup 1.0000x reference)
"""Multi-head attention (B=4, S=2048, E=768, H=12, D=64, causal) on 8 trn2
NeuronCores.

Sharding: core c -> batch b = c//2, head-half g = c%2 (6 heads each).
Each core computes its 6 heads' attention plus the partial output
projection; the host sums the two half-head partials per batch.

On-device strategy (per core), all matmul operands bf16 (PSUM f32):
  - Projections emit qk^T [f, s] (f on partitions, for scores) and
    V [k, f] (k on partitions, for ctx); contraction e chained over
    6 128-chunks.
  - Scores are computed transposed per k-chunk: S^T[k, q] = K^T Q,
    with masked-out leading q-columns never emitted.
  - exp on ACT per k-chunk over both heads ([128, 2x(512-c)] strided
    AP); causal diag blocks get a 0/1 strict-triangle multiply on DVE
    into a scratch tile (keeps masking off the PE and ACT).
  - ctx runs in [q, d] orientation: stationary = exp'd score block
    E^T[k, 128q], moving = V_aug [k, 65] (65th col of ones produces
    the softmax row-sums in the same chain).  65-col outputs make ctx
    ~2x cheaper on the PE than the [d, q] orientation.
  - Softmax normalization is a per-partition DVE reciprocal +
    tensor_scalar_mul (no PE broadcast needed in this orientation).
  - Normalized ctx [q, f] is PE-transposed per 128-block back to
    [f, q] for the output projection (chain of 3 f-chunks).
  - Engine balance: exp on ACT; norm + diag masks + V copies on DVE;
    qkT / ctxT / y PSUM->SBUF copies on GPSIMD; DMA on SP.
  - Emission interleaves the next head-pair's projections (and the
    output projections) into the attention u-loop so the PE never
    drains while ACT works through the exps.
"""
import sys, json, os

for _p in ("/opt/trn_rl_repo",):
    if _p not in sys.path and os.path.isdir(_p):
        sys.path.insert(0, _p)

import numpy as np
import concourse.bass as bass
import concourse.mybir as mybir
import concourse.tile as tile
from concourse.bass_utils import run_bass_kernel_spmd

B, S, E, H, D = 4, 2048, 768, 12, 64
HPC = H // 2          # heads per core = 6
FPC = HPC * D         # features per core per q/k/v = 384
EC = E // 128         # 6 contraction chunks for projections
SC = S // 128         # 16 s-chunks
QW = S // 512         # 4 q-windows
F32 = mybir.dt.float32
BF16 = mybir.dt.bfloat16
EXP = mybir.ActivationFunctionType.Exp
COPY = mybir.ActivationFunctionType.Copy
IDENT = mybir.ActivationFunctionType.Identity


def _patch_multiwait(nc, max_waits=1):
    """This container's walrus rejects instructions with more than one sync
    wait. Split excess waits onto same-engine NOPs emitted immediately
    before the instruction (same-engine streams are order-preserving)."""
    raw = nc.to_json_bytes()
    m = json.loads(raw)
    for f in m["functions"]:
        for b in f["blocks"]:
            out = []
            for inst in b["instructions"]:
                si = inst.get("sync_info") or {}
                ws = si.get("on_wait") or []
                if len(ws) > max_waits:
                    eng = inst["engine"]
                    for i, w in enumerate(ws[:-max_waits]):
                        out.append({
                            "debug": inst.get("debug", 0), "engine": eng,
                            "ins": [], "name": inst["name"] + f"-mw{i}",
                            "opcode": "NoOp", "outs": [],
                            "sync_info": {"on_update": [], "on_wait": [w]},
                        })
                    si["on_wait"] = ws[-max_waits:]
                out.append(inst)
            b["instructions"] = out
    patched = json.dumps(m).encode()
    nc.to_json_bytes = lambda: patched
    return nc


def build_nc(with_bias=False):
    nc = bass.Bass()
    xT = nc.dram_tensor("xT", [E, S], BF16, kind="ExternalInput")
    wqkT = nc.dram_tensor("wqkT", [E, 2 * FPC], BF16, kind="ExternalInput")
    wvT = nc.dram_tensor("wvT", [E, FPC], BF16, kind="ExternalInput")
    woT = nc.dram_tensor("woT", [FPC, E], BF16, kind="ExternalInput")
    ident = nc.dram_tensor("ident", [128, 128], BF16, kind="ExternalInput")
    trimask = nc.dram_tensor("trimask", [128, 128], BF16, kind="ExternalInput")
    if with_bias:
        bqk = nc.dram_tensor("bqk", [128, 6], F32, kind="ExternalInput")
        bv = nc.dram_tensor("bv", [1, FPC], BF16, kind="ExternalInput")
        bo = nc.dram_tensor("bo", [1, E], BF16, kind="ExternalInput")
    y = nc.dram_tensor("y", [S, E], F32, kind="ExternalOutput")

    with tile.TileContext(nc) as tc, \
         nc.allow_low_precision(reason="bf16 matmul pipeline by design"):
        with tc.tile_pool(name="persist", bufs=1) as P, \
             tc.tile_pool(name="ps", bufs=1, space="PSUM") as PS, \
             tc.tile_pool(name="esb", bufs=6) as EP, \
             tc.tile_pool(name="msk", bufs=6) as MP, \
             tc.tile_pool(name="nrm", bufs=8) as NP, \
             tc.tile_pool(name="osb", bufs=3) as OP:
            xT_sb = [P.tile([128, S], BF16, name=f"xT{i}") for i in range(EC)]
            wqkT_sb = [P.tile([128, 2 * FPC], BF16, name=f"wqkT{i}")
                       for i in range(EC)]
            wvT_sb = [P.tile([128, FPC], BF16, name=f"wvT{i}")
                      for i in range(EC)]
            woT_sb = [P.tile([128, E], BF16, name=f"woT{i}") for i in range(3)]
            qkT_sb = [P.tile([128, S], BF16, name=f"qkT{i}") for i in range(6)]
            V_sb = [P.tile([128, 65 * HPC], BF16, name=f"V{i}")
                    for i in range(SC)]
            ctxn_sb = [P.tile([128, FPC], BF16, name=f"ctxn{i}")
                       for i in range(SC)]
            ctxT_sb = [P.tile([128, S], BF16, name=f"ctxT{i}")
                       for i in range(3)]
            id_sb = P.tile([128, 128], BF16, name="id_sb")
            tm_sb = P.tile([128, 128], BF16, name="tm_sb")
            if with_bias:
                bqk_sb = P.tile([128, 6], F32, name="bqk_sb")
                bv_sb = P.tile([1, FPC], BF16, name="bv_sb")
                bo_sb = P.tile([1, E], BF16, name="bo_sb")
                on_sb = P.tile([1, 128], BF16, name="on_sb")
                nc.gpsimd.memset(on_sb[:], 1.0)

            def ps_tile(shape, tag, bufs):
                return PS.tile(shape, F32, name=tag, tag=tag, bufs=bufs)

            # ---------------- DMA in (arrival order = first-use order)
            for i in range(EC):
                nc.sync.dma_start(wvT_sb[i][:], wvT.ap()[128 * i:128 * (i + 1), :])
            for i in range(EC):
                nc.sync.dma_start(xT_sb[i][:, 0:512],
                                  xT.ap()[128 * i:128 * (i + 1), 0:512])
            for i in range(EC):
                nc.sync.dma_start(wqkT_sb[i][:],
                                  wqkT.ap()[128 * i:128 * (i + 1), :])
            for sw in (1, 2, 3):
                for i in range(EC):
                    nc.sync.dma_start(
                        xT_sb[i][:, 512 * sw:512 * (sw + 1)],
                        xT.ap()[128 * i:128 * (i + 1), 512 * sw:512 * (sw + 1)])
            for i in range(3):
                nc.sync.dma_start(woT_sb[i][:], woT.ap()[128 * i:128 * (i + 1), :])
            nc.sync.dma_start(id_sb[:], ident.ap())
            nc.sync.dma_start(tm_sb[:], trimask.ap())
            if with_bias:
                nc.sync.dma_start(bqk_sb[:], bqk.ap())
                nc.sync.dma_start(bv_sb[:], bv.ap())
                nc.sync.dma_start(bo_sb[:], bo.ap())

            # ---------------- projection emitters
            def emit_v_chain(hp, ki):
                """V [k=128, 2x64] for head pair hp at k-chunk ki."""
                psv = ps_tile([128, 448], "aux", 2)
                for ecc in range(EC):
                    nc.tensor.matmul(
                        psv[:, 0:128],
                        xT_sb[ecc][:, 128 * ki:128 * (ki + 1)],
                        wvT_sb[ecc][:, 128 * hp:128 * (hp + 1)],
                        start=(ecc == 0),
                        stop=(not with_bias and ecc == EC - 1),
                        skip_group_check=True)
                if with_bias:
                    nc.tensor.matmul(psv[:, 0:128], on_sb[:, 0:128],
                                     bv_sb[:, 128 * hp:128 * (hp + 1)],
                                     start=False, stop=True,
                                     skip_group_check=True)
                vv = V_sb[ki][:].rearrange("p (h x) -> p h x", x=65)
                nc.vector.tensor_copy(
                    vv[:, 2 * hp:2 * hp + 2, 0:64],
                    psv[:, 0:128].rearrange("p (h x) -> p h x", x=64))
                if hp == 0:
                    nc.gpsimd.memset(vv[:, :, 64:65], 1.0)

            def emit_qk_chain(hp, which, pair):
                """qkT chunk fo (q or k of head pair hp), s-windows pair."""
                fo = hp if which == 0 else 3 + hp
                psq = ps_tile([128, 1024], "pss", 2)
                for ecc in range(EC):
                    for h2 in range(2):
                        sw = 2 * pair + h2
                        nc.tensor.matmul(
                            psq[:, 512 * h2:512 * (h2 + 1)],
                            wqkT_sb[ecc][:, 128 * fo:128 * (fo + 1)],
                            xT_sb[ecc][:, 512 * sw:512 * (sw + 1)],
                            start=(ecc == 0), stop=(ecc == EC - 1),
                            skip_group_check=True)
                dst = qkT_sb[fo][:, 1024 * pair:1024 * (pair + 1)]
                if with_bias:
                    nc.scalar.activation(dst, psq[:], IDENT,
                                         bias=bqk_sb[:, fo:fo + 1])
                else:
                    nc.gpsimd.tensor_copy(dst, psq[:])

            def proj_items(hp):
                items = []
                for kg in range(4):
                    for ki in range(4 * kg, 4 * kg + 4):
                        items.append((emit_v_chain, hp, ki))
                    items.append((emit_qk_chain, hp, kg % 2, kg // 2))
                return items

            # ---------------- attention emitters
            def emit_scores(hp, qw, ki):
                """Scores + exp for k-chunk ki, both heads of pair hp.
                Returns (Et, masks) where Et[:, 512*hd + 128*qq ...] is the
                exp'd score block and masks[hd] the diag-masked copy."""
                qT, kT = qkT_sb[hp], qkT_sb[3 + hp]
                j = ki - 4 * qw
                c = 128 * j if j > 0 else 0
                pss = ps_tile([128, 1024], "pss", 2)
                Et = EP.tile([128, 1024], BF16, name="Et_t")
                for hd in range(2):
                    nc.tensor.matmul(
                        pss[:, 512 * hd + c:512 * (hd + 1)],
                        kT[64 * hd:64 * (hd + 1), 128 * ki:128 * (ki + 1)],
                        qT[64 * hd:64 * (hd + 1),
                           512 * qw + c:512 * (qw + 1)],
                        start=True, stop=True, skip_group_check=True)
                pv = pss[:].rearrange("p (h q) -> p h q", q=512)
                ev = Et[:].rearrange("p (h q) -> p h q", q=512)
                nc.scalar.activation(ev[:, :, c:512], pv[:, :, c:512],
                                     EXP, scale=0.125)
                masks = None
                if 0 <= j < 4:
                    masks = []
                    for hd in range(2):
                        Em = MP.tile([128, 128], BF16, name="Em_t")
                        nc.vector.tensor_mul(
                            Em[:],
                            Et[:, 512 * hd + 128 * j:512 * hd + 128 * (j + 1)],
                            tm_sb[:])
                        masks.append(Em)
                return Et, masks

            def emit_ctx(hp, qw, ki, Et, masks, psc):
                """ctx accumulation for k-chunk ki into [q, d] accumulators.
                Returns list of completed (hd, qq) pairs (diag reached)."""
                j = ki - 4 * qw
                done = []
                for hd in range(2):
                    hh = 2 * hp + hd
                    for qq in range(max(j, 0), 4):
                        stat = masks[hd] if qq == j else \
                            Et[:, 512 * hd + 128 * qq:512 * hd + 128 * (qq + 1)]
                        nc.tensor.matmul(
                            psc[hd][:, 65 * qq:65 * (qq + 1)],
                            stat, V_sb[ki][:, 65 * hh:65 * (hh + 1)],
                            start=(ki == 0), stop=(qq == j),
                            skip_group_check=True)
                        if qq == j:
                            done.append((hd, qq))
                return done

            def emit_norm(hp, qw, psc, hd, qq):
                sc = 4 * qw + qq
                hh = 2 * hp + hd
                rinv = NP.tile([128, 1], F32, name="rinv_t")
                nc.vector.reciprocal(rinv[:],
                                     psc[hd][:, 65 * qq + 64:65 * qq + 65])
                nc.vector.tensor_scalar_mul(
                    ctxn_sb[sc][:, 64 * hh:64 * (hh + 1)],
                    psc[hd][:, 65 * qq:65 * qq + 64], rinv[:])

            def emit_transpose(hp, sc):
                pt = ps_tile([128, 448], "aux", 2)
                ptb = pt[:, 0:64].bitcast(BF16)
                nc.tensor.matmul(ptb, ctxn_sb[sc][:, 128 * hp:128 * (hp + 1)],
                                 id_sb[:], is_transpose=True)
                nc.gpsimd.tensor_copy(ctxT_sb[hp][:, 128 * sc:128 * (sc + 1)],
                                      ptb)

            def emit_outproj(sc):
                pos = [ps_tile([128, 448], "aux", 2) for _ in range(2)]
                for cc in range(3):
                    for eh in range(2):
                        nc.tensor.matmul(
                            pos[eh][:, 0:384],
                            ctxT_sb[cc][:, 128 * sc:128 * (sc + 1)],
                            woT_sb[cc][:, 384 * eh:384 * (eh + 1)],
                            start=(cc == 0),
                            stop=(not with_bias and cc == 2),
                            skip_group_check=True)
                ysb = OP.tile([128, E], F32, name="ysb_t")
                for eh in range(2):
                    if with_bias:
                        nc.tensor.matmul(pos[eh][:, 0:384], on_sb[:, 0:128],
                                         bo_sb[:, 384 * eh:384 * (eh + 1)],
                                         start=False, stop=True,
                                         skip_group_check=True)
                    nc.gpsimd.tensor_copy(ysb[:, 384 * eh:384 * (eh + 1)],
                                          pos[eh][:, 0:384])
                nc.sync.dma_start(y.ap()[128 * sc:128 * (sc + 1), :], ysb[:])

            # ---------------- pipelined emission
            def attn(hp, filler):
                pending = None      # (qw, ki, Et, masks, psc)
                pendingT = []       # transposes deferred one step
                ready_out = []      # per-sc outproj (hp == 2 only)

                def flush():
                    nonlocal pending
                    for t_sc in pendingT:
                        emit_transpose(hp, t_sc)
                        if hp == 2:
                            ready_out.append(t_sc)
                    pendingT.clear()
                    if pending is not None:
                        fqw, fki, fEt, fmasks, fpsc = pending
                        done = emit_ctx(hp, fqw, fki, fEt, fmasks, fpsc)
                        for hd, qq in done:
                            emit_norm(hp, fqw, fpsc, hd, qq)
                        for hd, qq in done:
                            if hd == 1:
                                pendingT.append(4 * fqw + qq)
                        pending = None

                def step_extras():
                    if filler:
                        fn, *args = filler.pop(0)
                        fn(*args)
                    if ready_out:
                        emit_outproj(ready_out.pop(0))

                for qw in range(QW):
                    psc = [ps_tile([128, 260], "pscA", 1),
                           ps_tile([128, 260], "pscB", 1)]
                    for ki in range(4 * qw + 4):
                        Et, masks = emit_scores(hp, qw, ki)
                        flush()
                        step_extras()
                        pending = (qw, ki, Et, masks, psc)
                flush()
                flush()   # drain deferred transposes
                while filler or ready_out:
                    step_extras()

            for it in proj_items(0):
                fn, *args = it
                fn(*args)
            attn(0, proj_items(1))
            attn(1, proj_items(2))
            attn(2, [])

    return _patch_multiwait(nc)


_NC = {}


def _get_nc(with_bias=False):
    if with_bias not in _NC:
        _NC[with_bias] = build_nc(with_bias=with_bias)
    return _NC[with_bias]


def _prep_core_inputs(x, in_proj_w, in_proj_b, out_w, out_b, with_bias):
    """Build the 8 per-core input dicts (host-side shard + transpose)."""
    import ml_dtypes
    BF = ml_dtypes.bfloat16
    id_bf = np.eye(128, dtype=np.float32).astype(BF)
    # E^T block [k rows, q cols]: keep k <= q
    tm_bf = (np.arange(128)[:, None] <= np.arange(128)[None, :]) \
        .astype(np.float32).astype(BF)
    xT_by_b = [np.ascontiguousarray(np.asarray(x[b]).T.astype(BF))
               for b in range(B)]

    in_maps = []
    for c in range(8):
        b = c // 2
        g = c % 2
        f0 = FPC * g
        Wq = np.asarray(in_proj_w[f0:f0 + FPC])
        Wk = np.asarray(in_proj_w[E + f0:E + f0 + FPC])
        Wv = np.asarray(in_proj_w[2 * E + f0:2 * E + f0 + FPC])
        Wo = np.asarray(out_w[:, f0:f0 + FPC])
        d = {
            "xT": xT_by_b[b],
            "wqkT": np.ascontiguousarray(
                np.concatenate([Wq, Wk], axis=0).T.astype(BF)),
            "wvT": np.ascontiguousarray(Wv.T.astype(BF)),
            "woT": np.ascontiguousarray(Wo.T.astype(BF)),
            "ident": id_bf,
            "trimask": tm_bf,
        }
        if with_bias:
            bq = np.asarray(in_proj_b[f0:f0 + FPC])
            bk = np.asarray(in_proj_b[E + f0:E + f0 + FPC])
            bvv = np.asarray(in_proj_b[2 * E + f0:2 * E + f0 + FPC])
            d["bqk"] = np.ascontiguousarray(
                np.concatenate([bq, bk]).astype(np.float32).reshape(6, 128).T)
            d["bv"] = bvv.reshape(1, FPC).astype(BF)
            # out bias only on even cores so the host-side pair-sum is exact
            d["bo"] = (np.asarray(out_b).reshape(1, E).astype(BF) if g == 0
                       else np.zeros((1, E), BF))
        in_maps.append(d)
    return in_maps


def kernel(x, in_proj_w, in_proj_b, out_w, out_b):
    with_bias = bool(np.any(np.asarray(in_proj_b))) or \
                bool(np.any(np.asarray(out_b)))
    nc = _get_nc(with_bias=with_bias)
    in_maps = _prep_core_inputs(x, in_proj_w, in_proj_b, out_w, out_b,
                                with_bias)
    res = run_bass_kernel_spmd(nc, in_maps, core_ids=list(range(8)))
    out = np.empty((B, S, E), np.float32)
    for b in range(B):
        out[b] = res.results[2 * b]["y"] + res.results[2 * b + 1]["y"]
    return out


# revision 3
# speedup vs baseline: 1.0750x; 1.0750x over previous
"""Multi-head attention (B=4, S=2048, E=768, H=12, D=64, causal) on 8 trn2
NeuronCores.

Sharding: core c -> batch b = c//2, head-half g = c%2 (6 heads each).
Each core computes its 6 heads' attention plus the partial output
projection; the host sums the two half-head partials per batch.

On-device strategy (per core), all matmul operands bf16 (PSUM f32):
  - Projections emit qk^T [f, s] (f on partitions, for scores) and
    V [k, f] (k on partitions, for ctx); contraction e chained over
    6 128-chunks.
  - Scores are computed transposed per k-chunk: S^T[k, q] = K^T Q,
    with masked-out leading q-columns never emitted.
  - exp on ACT per k-chunk over both heads ([128, 2x(512-c)] strided
    AP); causal diag blocks get a 0/1 triangle multiply on DVE into a
    scratch tile (keeps masking off the PE and ACT).
  - ctx runs in [q, d] orientation: stationary = exp'd score block
    E^T[k, 128q], moving = V_aug [k, 65] (65th col of ones produces
    the softmax row-sums in the same chain).  65-col outputs make ctx
    ~2x cheaper on the PE than the [d, q] orientation.
  - Softmax normalization is a per-partition DVE reciprocal +
    tensor_scalar_mul (no PE broadcast needed in this orientation).
  - Normalized ctx [q, f] is PE-transposed per 128-block back to
    [f, q] for the output projection (chain of 3 f-chunks).
  - Engine balance: exp on ACT; norm + diag masks + V copies on DVE;
    qkT / ctxT / y PSUM->SBUF copies on GPSIMD; DMA on SP.
  - Inputs land in 8 batched DMAs (the per-DMA DGE setup time on SP
    otherwise serializes the start); arrival order matches first use.
  - Emission pipeline: ctx trails scores by two k-chunks so ACT's exp
    and DVE's diag mask always land before the PE needs them as
    stationary; projections for the next head pair (and the output
    projections) are interleaved into the u-loop as fillers.
"""
import sys, json, os

for _p in ("/opt/trn_rl_repo",):
    if _p not in sys.path and os.path.isdir(_p):
        sys.path.insert(0, _p)

import numpy as np
import concourse.bass as bass
import concourse.mybir as mybir
import concourse.tile as tile
from concourse.bass_utils import run_bass_kernel_spmd

B, S, E, H, D = 4, 2048, 768, 12, 64
HPC = H // 2          # heads per core = 6
FPC = HPC * D         # features per core per q/k/v = 384
EC = E // 128         # 6 contraction chunks for projections
SC = S // 128         # 16 s-chunks
QW = S // 512         # 4 q-windows
F32 = mybir.dt.float32
BF16 = mybir.dt.bfloat16
EXP = mybir.ActivationFunctionType.Exp
IDENT = mybir.ActivationFunctionType.Identity
DEPTH = 2             # ctx trails scores by this many k-chunks


def _patch_multiwait(nc, max_waits=1):
    """This container's walrus rejects instructions with more than one sync
    wait. Split excess waits onto same-engine NOPs emitted immediately
    before the instruction (same-engine streams are order-preserving)."""
    raw = nc.to_json_bytes()
    m = json.loads(raw)
    for f in m["functions"]:
        for b in f["blocks"]:
            out = []
            for inst in b["instructions"]:
                si = inst.get("sync_info") or {}
                ws = si.get("on_wait") or []
                if len(ws) > max_waits:
                    eng = inst["engine"]
                    for i, w in enumerate(ws[:-max_waits]):
                        out.append({
                            "debug": inst.get("debug", 0), "engine": eng,
                            "ins": [], "name": inst["name"] + f"-mw{i}",
                            "opcode": "NoOp", "outs": [],
                            "sync_info": {"on_update": [], "on_wait": [w]},
                        })
                    si["on_wait"] = ws[-max_waits:]
                out.append(inst)
            b["instructions"] = out
    patched = json.dumps(m).encode()
    nc.to_json_bytes = lambda: patched
    return nc


def build_nc(with_bias=False):
    nc = bass.Bass()
    # host-side layouts (see _prep_core_inputs):
    #   xT   [128, sw, ecc, 512] : x[b].T chunked by s-window then e-chunk
    #   wqkT [128, ecc, 768]     : (Wq|Wk).T per e-chunk (f = q 0:384, k 384:768)
    #   wvT  [128, ecc, 384]     : Wv.T per e-chunk
    #   woT  [128, fc, 768]      : Wo.T per f-chunk
    #   idtm [128, 256]          : identity | strict-upper 0/1 triangle
    xT = nc.dram_tensor("xT", [128, QW * EC * 512], BF16, kind="ExternalInput")
    wqkT = nc.dram_tensor("wqkT", [128, EC * 768], BF16, kind="ExternalInput")
    wvT = nc.dram_tensor("wvT", [128, EC * FPC], BF16, kind="ExternalInput")
    woT = nc.dram_tensor("woT", [128, 3 * E], BF16, kind="ExternalInput")
    idtm = nc.dram_tensor("idtm", [128, 256], BF16, kind="ExternalInput")
    if with_bias:
        bqk = nc.dram_tensor("bqk", [128, 6], F32, kind="ExternalInput")
        bv = nc.dram_tensor("bv", [1, FPC], BF16, kind="ExternalInput")
        bo = nc.dram_tensor("bo", [1, E], BF16, kind="ExternalInput")
    y = nc.dram_tensor("y", [S, E], F32, kind="ExternalOutput")

    with tile.TileContext(nc) as tc, \
         nc.allow_low_precision(reason="bf16 matmul pipeline by design"):
        with tc.tile_pool(name="persist", bufs=1) as P, \
             tc.tile_pool(name="ps", bufs=1, space="PSUM") as PS, \
             tc.tile_pool(name="esb", bufs=6) as EP, \
             tc.tile_pool(name="msk", bufs=8) as MP, \
             tc.tile_pool(name="nrm", bufs=8) as NP, \
             tc.tile_pool(name="osb", bufs=3) as OP:
            xT_all = P.tile([128, EC * S], BF16, name="xT_all")
            wqkT_all = P.tile([128, EC * 768], BF16, name="wqkT_all")
            wvT_all = P.tile([128, EC * FPC], BF16, name="wvT_all")
            woT_all = P.tile([128, 3 * E], BF16, name="woT_all")
            idtm_sb = P.tile([128, 256], BF16, name="idtm_sb")
            qkT_sb = [P.tile([128, S], BF16, name=f"qkT{i}") for i in range(6)]
            V_sb = [P.tile([128, 65 * HPC], BF16, name=f"V{i}")
                    for i in range(SC)]
            ctxn_sb = [P.tile([128, FPC], BF16, name=f"ctxn{i}")
                       for i in range(SC)]
            ctxT_sb = [P.tile([128, S], BF16, name=f"ctxT{i}")
                       for i in range(3)]
            # e-chunk accessors
            xT_c = [xT_all[:, S * i:S * (i + 1)] for i in range(EC)]
            wqkT_c = [wqkT_all[:, 768 * i:768 * (i + 1)] for i in range(EC)]
            wvT_c = [wvT_all[:, FPC * i:FPC * (i + 1)] for i in range(EC)]
            woT_c = [woT_all[:, E * i:E * (i + 1)] for i in range(3)]
            id_sb = idtm_sb[:, 0:128]
            tm_sb = idtm_sb[:, 128:256]
            if with_bias:
                bqk_sb = P.tile([128, 6], F32, name="bqk_sb")
                bv_sb = P.tile([1, FPC], BF16, name="bv_sb")
                bo_sb = P.tile([1, E], BF16, name="bo_sb")
                on_sb = P.tile([1, 128], BF16, name="on_sb")
                nc.gpsimd.memset(on_sb[:], 1.0)

            def ps_tile(shape, tag, bufs):
                return PS.tile(shape, F32, name=tag, tag=tag, bufs=bufs)

            # ---------------- DMA in (arrival order = first-use order)
            nc.sync.dma_start(wvT_all[:], wvT.ap())
            nc.sync.dma_start(idtm_sb[:], idtm.ap())
            xv = xT_all[:].rearrange("p (e s) -> p e s", s=S)
            for sw in range(QW):
                nc.sync.dma_start(
                    xv[:, :, 512 * sw:512 * (sw + 1)],
                    xT.ap()[:, EC * 512 * sw:EC * 512 * (sw + 1)]
                    .rearrange("p (e s) -> p e s", s=512))
                if sw == 0:
                    nc.sync.dma_start(wqkT_all[:], wqkT.ap())
            nc.sync.dma_start(woT_all[:], woT.ap())
            if with_bias:
                nc.sync.dma_start(bqk_sb[:], bqk.ap())
                nc.sync.dma_start(bv_sb[:], bv.ap())
                nc.sync.dma_start(bo_sb[:], bo.ap())

            # ---------------- projection emitters
            def emit_v_chain(hp, ki):
                """V [k=128, 2x64] for head pair hp at k-chunk ki."""
                psv = ps_tile([128, 448], "aux", 2)
                for ecc in range(EC):
                    nc.tensor.matmul(
                        psv[:, 0:128],
                        xT_c[ecc][:, 128 * ki:128 * (ki + 1)],
                        wvT_c[ecc][:, 128 * hp:128 * (hp + 1)],
                        start=(ecc == 0),
                        stop=(not with_bias and ecc == EC - 1),
                        skip_group_check=True)
                if with_bias:
                    nc.tensor.matmul(psv[:, 0:128], on_sb[:, 0:128],
                                     bv_sb[:, 128 * hp:128 * (hp + 1)],
                                     start=False, stop=True,
                                     skip_group_check=True)
                vv = V_sb[ki][:].rearrange("p (h x) -> p h x", x=65)
                nc.vector.tensor_copy(
                    vv[:, 2 * hp:2 * hp + 2, 0:64],
                    psv[:, 0:128].rearrange("p (h x) -> p h x", x=64))
                if hp == 0:
                    nc.gpsimd.memset(vv[:, :, 64:65], 1.0)

            def emit_qk_chain(hp, which, sw):
                """qkT chunk fo (q or k of head pair hp), one s-window."""
                fo = hp if which == 0 else 3 + hp
                psq = ps_tile([128, 1024], "pss", 2)
                for ecc in range(EC):
                    nc.tensor.matmul(
                        psq[:, 0:512],
                        wqkT_c[ecc][:, 128 * fo:128 * (fo + 1)],
                        xT_c[ecc][:, 512 * sw:512 * (sw + 1)],
                        start=(ecc == 0), stop=(ecc == EC - 1),
                        skip_group_check=True)
                dst = qkT_sb[fo][:, 512 * sw:512 * (sw + 1)]
                if with_bias:
                    nc.scalar.activation(dst, psq[:, 0:512], IDENT,
                                         bias=bqk_sb[:, fo:fo + 1])
                else:
                    nc.gpsimd.tensor_copy(dst, psq[:, 0:512])

            def proj_items(hp):
                items = []
                for kg in range(4):
                    for ki in range(4 * kg, 4 * kg + 4):
                        items.append((emit_v_chain, hp, ki))
                    items.append((emit_qk_chain, hp, 0, kg))
                    items.append((emit_qk_chain, hp, 1, kg))
                return items

            # ---------------- attention emitters
            def emit_scores(hp, qw, ki):
                """Scores + exp for k-chunk ki, both heads of pair hp.
                Returns (Et, masks) where Et[:, 512*hd + 128*qq ...] is the
                exp'd score block and masks[hd] the diag-masked copy."""
                qT, kT = qkT_sb[hp], qkT_sb[3 + hp]
                j = ki - 4 * qw
                c = 128 * j if j > 0 else 0
                pss = ps_tile([128, 1024], "pss", 2)
                Et = EP.tile([128, 1024], BF16, name="Et_t")
                for hd in range(2):
                    nc.tensor.matmul(
                        pss[:, 512 * hd + c:512 * (hd + 1)],
                        kT[64 * hd:64 * (hd + 1), 128 * ki:128 * (ki + 1)],
                        qT[64 * hd:64 * (hd + 1),
                           512 * qw + c:512 * (qw + 1)],
                        start=True, stop=True, skip_group_check=True)
                pv = pss[:].rearrange("p (h q) -> p h q", q=512)
                ev = Et[:].rearrange("p (h q) -> p h q", q=512)
                nc.scalar.activation(ev[:, :, c:512], pv[:, :, c:512],
                                     EXP, scale=0.125)
                masks = None
                if 0 <= j < 4:
                    masks = []
                    for hd in range(2):
                        Em = MP.tile([128, 128], BF16, name="Em_t")
                        nc.vector.tensor_mul(
                            Em[:],
                            Et[:, 512 * hd + 128 * j:512 * hd + 128 * (j + 1)],
                            tm_sb)
                        masks.append(Em)
                return Et, masks

            def emit_ctx(hp, qw, ki, Et, masks, psc):
                """ctx accumulation for k-chunk ki into [q, d] accumulators.
                Returns list of completed (hd, qq) pairs (diag reached)."""
                j = ki - 4 * qw
                done = []
                for hd in range(2):
                    hh = 2 * hp + hd
                    for qq in range(max(j, 0), 4):
                        stat = masks[hd] if qq == j else \
                            Et[:, 512 * hd + 128 * qq:512 * hd + 128 * (qq + 1)]
                        nc.tensor.matmul(
                            psc[hd][:, 65 * qq:65 * (qq + 1)],
                            stat, V_sb[ki][:, 65 * hh:65 * (hh + 1)],
                            start=(ki == 0), stop=(qq == j),
                            skip_group_check=True)
                        if qq == j:
                            done.append((hd, qq))
                return done

            def emit_norm(hp, qw, psc, hd, qq):
                sc = 4 * qw + qq
                hh = 2 * hp + hd
                rinv = NP.tile([128, 1], F32, name="rinv_t")
                nc.vector.reciprocal(rinv[:],
                                     psc[hd][:, 65 * qq + 64:65 * qq + 65])
                nc.vector.tensor_scalar_mul(
                    ctxn_sb[sc][:, 64 * hh:64 * (hh + 1)],
                    psc[hd][:, 65 * qq:65 * qq + 64], rinv[:])

            def emit_transpose(hp, sc):
                pt = ps_tile([128, 448], "aux", 2)
                ptb = pt[:, 0:64].bitcast(BF16)
                nc.tensor.matmul(ptb, ctxn_sb[sc][:, 128 * hp:128 * (hp + 1)],
                                 id_sb, is_transpose=True)
                nc.gpsimd.tensor_copy(ctxT_sb[hp][:, 128 * sc:128 * (sc + 1)],
                                      ptb)

            def emit_outproj(sc):
                pos = [ps_tile([128, 448], "aux", 2) for _ in range(2)]
                for cc in range(3):
                    for eh in range(2):
                        nc.tensor.matmul(
                            pos[eh][:, 0:384],
                            ctxT_sb[cc][:, 128 * sc:128 * (sc + 1)],
                            woT_c[cc][:, 384 * eh:384 * (eh + 1)],
                            start=(cc == 0),
                            stop=(not with_bias and cc == 2),
                            skip_group_check=True)
                ysb = OP.tile([128, E], F32, name="ysb_t")
                for eh in range(2):
                    if with_bias:
                        nc.tensor.matmul(pos[eh][:, 0:384], on_sb[:, 0:128],
                                         bo_sb[:, 384 * eh:384 * (eh + 1)],
                                         start=False, stop=True,
                                         skip_group_check=True)
                    nc.gpsimd.tensor_copy(ysb[:, 384 * eh:384 * (eh + 1)],
                                          pos[eh][:, 0:384])
                nc.sync.dma_start(y.ap()[128 * sc:128 * (sc + 1), :], ysb[:])

            # ---------------- pipelined emission
            def attn(hp, filler):
                pending = []        # [(qw, ki, Et, masks, psc), ...]
                pendingT = []       # transposes deferred one step
                ready_out = []      # per-sc outproj (hp == 2 only)
                nsteps = sum(2 * qw + 2 for qw in range(QW)) * 2  # 40 kis

                def flush_one():
                    fqw, fki, fEt, fmasks, fpsc = pending.pop(0)
                    done = emit_ctx(hp, fqw, fki, fEt, fmasks, fpsc)
                    for hd, qq in done:
                        emit_norm(hp, fqw, fpsc, hd, qq)
                    for hd, qq in done:
                        if hd == 1:
                            pendingT.append(4 * fqw + qq)

                def step_extras(steps_left):
                    while pendingT:
                        t_sc = pendingT.pop(0)
                        emit_transpose(hp, t_sc)
                        if hp == 2:
                            ready_out.append(t_sc)
                    npop = 1
                    if filler and steps_left > 0 and \
                            len(filler) > steps_left:
                        npop = 2
                    for _ in range(npop):
                        if filler:
                            fn, *args = filler.pop(0)
                            fn(*args)
                    if ready_out:
                        emit_outproj(ready_out.pop(0))

                step = 0
                for qw in range(QW):
                    psc = [ps_tile([128, 260], "pscA", 1),
                           ps_tile([128, 260], "pscB", 1)]
                    for ki in range(4 * qw + 4):
                        Et, masks = emit_scores(hp, qw, ki)
                        pending.append((qw, ki, Et, masks, psc))
                        if len(pending) > DEPTH:
                            flush_one()
                        step += 1
                        step_extras(40 - step)
                while pending:
                    flush_one()
                    step_extras(0)
                step_extras(0)
                while filler or ready_out or pendingT:
                    step_extras(0)

            # minimal lead-in: V k-chunks 0-3 and the first q/k s-window,
            # everything else rides the attention u-loops as fillers
            lead = proj_items(0)
            head = [lead[i] for i in (0, 1, 2, 3, 4, 5)]
            rest = lead[6:]
            for fn, *args in head:
                fn(*args)
            attn(0, rest + proj_items(1))
            attn(1, proj_items(2))
            attn(2, [])

    return _patch_multiwait(nc)


_NC = {}


def _get_nc(with_bias=False):
    if with_bias not in _NC:
        _NC[with_bias] = build_nc(with_bias=with_bias)
    return _NC[with_bias]


def _prep_core_inputs(x, in_proj_w, in_proj_b, out_w, out_b, with_bias):
    """Build the 8 per-core input dicts (host-side shard + transpose)."""
    import ml_dtypes
    BF = ml_dtypes.bfloat16
    id_np = np.eye(128, dtype=np.float32)
    # E^T block [k rows, q cols]: keep k <= q
    tm_np = (np.arange(128)[:, None] <= np.arange(128)[None, :]) \
        .astype(np.float32)
    idtm = np.ascontiguousarray(
        np.concatenate([id_np, tm_np], axis=1).astype(BF))

    def chunk128(a):
        """[E_rows, N] -> [128, n_chunks * N] with chunks along columns."""
        n = a.shape[0] // 128
        return np.ascontiguousarray(
            a.reshape(n, 128, a.shape[1]).transpose(1, 0, 2)
            .reshape(128, -1).astype(BF))

    xT_by_b = []
    for b in range(B):
        a = np.asarray(x[b]).T.reshape(EC, 128, QW, 512)  # [ecc, p, sw, 512]
        xT_by_b.append(np.ascontiguousarray(
            a.transpose(1, 2, 0, 3).reshape(128, -1).astype(BF)))

    in_maps = []
    for c in range(8):
        b = c // 2
        g = c % 2
        f0 = FPC * g
        Wq = np.asarray(in_proj_w[f0:f0 + FPC])
        Wk = np.asarray(in_proj_w[E + f0:E + f0 + FPC])
        Wv = np.asarray(in_proj_w[2 * E + f0:2 * E + f0 + FPC])
        Wo = np.asarray(out_w[:, f0:f0 + FPC])
        d = {
            "xT": xT_by_b[b],
            "wqkT": chunk128(np.concatenate([Wq, Wk], axis=0).T),
            "wvT": chunk128(Wv.T),
            "woT": chunk128(Wo.T),
            "idtm": idtm,
        }
        if with_bias:
            bq = np.asarray(in_proj_b[f0:f0 + FPC])
            bk = np.asarray(in_proj_b[E + f0:E + f0 + FPC])
            bvv = np.asarray(in_proj_b[2 * E + f0:2 * E + f0 + FPC])
            d["bqk"] = np.ascontiguousarray(
                np.concatenate([bq, bk]).astype(np.float32).reshape(6, 128).T)
            d["bv"] = bvv.reshape(1, FPC).astype(BF)
            # out bias only on even cores so the host-side pair-sum is exact
            d["bo"] = (np.asarray(out_b).reshape(1, E).astype(BF) if g == 0
                       else np.zeros((1, E), BF))
        in_maps.append(d)
    return in_maps


def kernel(x, in_proj_w, in_proj_b, out_w, out_b):
    with_bias = bool(np.any(np.asarray(in_proj_b))) or \
                bool(np.any(np.asarray(out_b)))
    nc = _get_nc(with_bias=with_bias)
    in_maps = _prep_core_inputs(x, in_proj_w, in_proj_b, out_w, out_b,
                                with_bias)
    res = run_bass_kernel_spmd(nc, in_maps, core_ids=list(range(8)))
    out = np.empty((B, S, E), np.float32)
    for b in range(B):
        out[b] = res.results[2 * b]["y"] + res.results[2 * b + 1]["y"]
    return out
